# revision 11
# baseline (speedup 1.0000x reference)
"""Trainium2 Bass kernel for nn_DynamicFeatureGroupingLayer.

Reference computation (B=4096, G=10 groups of S=100 features, M=4 masks,
H=512 hidden):
    mask = entmax(1.1, W_masks)                       # [G,M,S]
    h_t[b,g,m,:] = (x_g[b] * mask[g,m]) @ W_t[g].T    # t in {1,2}
    n_t = layernorm(h_t)
    out[b,g] = sum_m relu(sigmoid(n_1) * n_2)         # [B, G*H]

Strategy:
  * Data-parallel over batch across 8 cores (512 rows each).
  * Host folds the mask into the weights: W~_t[g,m] = mask[g,m,:] * W_t[g]
    so h_t = x_g @ W~_t.T is a plain matmul (K=S=100, stationary = x chunk).
  * LN means come free as extra matmul columns (sum_h W~/H).
  * LN second moments via a Cholesky-Gram trick: ss = ||L^T x||^2 with
    L = chol(W~^T W~) [S,S]; stats run in x-space (S=100), 4x cheaper than
    h-space (H=512). Sum-of-squares via DVE bn_stats (even/odd mean+M2
    per 100-col chunk), recombined in the per-block smalls.
  * Epilogue per (group, mask): one ACT Sigmoid with per-partition
    scale/bias (rs1, nb1=-mu1*rs1), then a single custom DVE op
    GATE = relu(h2*rs2 + nb2) * s  ==  sigmoid(n1) * relu(n2)
    (valid since s > 0); mask-sum as 3 bf16 adds on GPSIMD; y is bf16.
"""

import numpy as np

B = 4096
INPUT_SIZE = 1000
H = 512
M = 4
S = 100
G = 10
N_CORES = 8
BC = B // N_CORES            # batch rows per core (512)
NBC = BC // 128              # 128-row chunks per core (4)
GRP = 5                      # groups g per block
EPS_LN = 1e-5

MM_DTYPE = "bf16"

_STATE = {}


# --------------------------------------------------------------------------
# host-side preprocessing
# --------------------------------------------------------------------------

def _entmax(alpha, v):
    v = v - np.max(v, axis=-1, keepdims=True)
    e = np.exp(v)
    s = (np.sum(e ** alpha, axis=-1, keepdims=True) + 1e-5) ** (1.0 / alpha)
    return e / s


def _host_prep(x, W_masks, W1, W2):
    """Returns (xt_per_core, W_rhs, L_rhs) as float32 arrays."""
    x = np.asarray(x, np.float32)
    mask = _entmax(1.1, np.asarray(W_masks, np.float64)).astype(np.float64)
    W1 = np.asarray(W1, np.float64)
    W2 = np.asarray(W2, np.float64)

    # W~_t[g,m,h,s] = mask[g,m,s] * W_t[g,h,s]
    Wt1 = mask[:, :, None, :] * W1[:, None, :, :]        # [G,M,H,S]
    Wt2 = mask[:, :, None, :] * W2[:, None, :, :]
    # main rhs: [G, S, M*2*H], col = m*1024 + t*512 + h
    W_rhs = np.stack([Wt1, Wt2], axis=2)                  # [G,M,2,H,S]
    W_rhs = W_rhs.transpose(0, 4, 1, 2, 3).reshape(G, S, M * 2 * H)

    # mean cols: value = sum_h W~/H; ride in spare tail cols 228:236 of the
    # m=0 block of L_rhs, order 2m+t
    MU = np.stack([Wt1.mean(axis=2), Wt2.mean(axis=2)], axis=2)  # [G,M,2,S]
    MU_rhs = MU.transpose(0, 3, 1, 2).reshape(G, S, 2 * M)

    # cholesky of gram matrices, tight-packed: chunk mt = m*2+t at col
    # (mt%4)*100 + (mt//4)*512; mu cols at 400:408 of the first half
    L_rhs = np.zeros((G, S, M * 256), np.float64)
    for g in range(G):
        for m in range(M):
            for t, Wt in enumerate((Wt1, Wt2)):
                Wm = Wt[g, m]                              # [H,S]
                Gm = Wm.T @ Wm                             # [S,S]
                jit = 1e-9 * np.trace(Gm) / S
                Lm = np.linalg.cholesky(Gm + jit * np.eye(S))
                mt = m * 2 + t
                base = (mt % 4) * 100 + (mt // 4) * 512
                L_rhs[g, :, base:base + S] = Lm
    L_rhs[:, :, 400:408] = MU_rhs

    # x transposed per core: xt[s, g*512 + b] = x[c*512+b, g*100+s]
    xt_cores = []
    for c in range(N_CORES):
        xc = x[c * BC:(c + 1) * BC]                        # [512, 1000]
        xt = np.ascontiguousarray(
            xc.reshape(BC, G, S).transpose(2, 1, 0).reshape(S, G * BC))
        xt_cores.append(xt)

    return xt_cores, W_rhs.astype(np.float32), L_rhs.astype(np.float32)


# --------------------------------------------------------------------------
# tile patch (this walrus build accepts at most ONE sync wait per inst)
# --------------------------------------------------------------------------

def _install_tile_patch():
    import concourse.mybir as mybir
    from concourse.tile import TileContext, ScopedClock

    if getattr(TileContext, "_drain_patched", False):
        return

    def _patched(self, tick_clock, wait_clock):
        nc = self.nc
        probe = nc.sync.nop(hint="drain_waits", nofuse=True)
        wait_clock.add_sem_waits(
            probe.ins, ScopedClock({None: tick_clock.global_clock}))
        si = probe.ins.sync_info
        if si is not None and len(si.on_wait) > 1:
            waits = list(si.on_wait)
            si.on_wait = [waits[0]]
            probe.ins.sync_info = si
            for w in waits[1:]:
                extra = nc.sync.nop(hint="drain_waits_x", nofuse=True)
                extra.ins.sync_info = mybir.SyncInfo(on_wait=[w], on_update=[])
        nc.sync.drain()
        nc.all_engine_barrier()
        popped = nc._tile_sem_poison_stack.pop()
        assert popped is self._sem_poison
        nc.clear_and_free_semaphores(list(self.sems.allocated().values()))
        nc.all_engine_barrier()

    TileContext._drain_and_barrier = _patched

    orig_commit = TileContext._commit_instruction

    def _commit_split(self, inst, lazy_reg_writes=True):
        si = inst.sync_info
        if (
            si is not None
            and len(si.on_wait) > 1
            and inst.engine != mybir.EngineType.Unassigned
        ):
            waits = list(si.on_wait)
            for w in waits[:-1]:
                nop = mybir.InstNoOp(
                    name=self.nc.get_next_instruction_name(),
                    engine=inst.engine,
                    ins=[],
                    outs=[],
                    sync_info=mybir.SyncInfo(on_wait=[w], on_update=[]),
                )
                orig_commit(self, nop, lazy_reg_writes=False)
            si.on_wait = [waits[-1]]
            inst.sync_info = si
        return orig_commit(self, inst, lazy_reg_writes)

    TileContext._commit_instruction = _commit_split
    TileContext._drain_patched = True


# --------------------------------------------------------------------------
# custom DVE op: GATE = relu(Src0*C0 + C1) * Src1
# (= sigmoid(n1) * relu(layernorm(h2)) with C0=rs2, C1=-mu2*rs2, Src1=s)
# --------------------------------------------------------------------------

_GATE_OP = None


def _register_gate_op():
    global _GATE_OP
    if _GATE_OP is not None:
        return _GATE_OP
    import concourse.dve_ops as dve_ops
    from concourse.dve_ops import DveOp, _dve_relu, _CUSTOM_DVE_ROW_BASE
    from concourse.dve_spec import C0, C1, Spec, Src0, Src1, lower, relu
    from concourse.dve_uop import DveOpSpec

    NAME = "TENSOR_GATE_LNRELU"
    for op in dve_ops.OPS:
        if op.name == NAME:
            _GATE_OP = op
            return op

    spec = Spec(
        body=relu(Src0 * C0 + C1) * Src1,
        reference=lambda in0, in1, c0, c1, c2: (
            _dve_relu(in0.astype(np.float32) * c0 + c1) * in1
        ),
    )
    row = _CUSTOM_DVE_ROW_BASE + len(dve_ops.OPS)
    shas = {}
    for ver in ("v3", "v4"):
        try:
            shas[ver] = DveOpSpec(
                name=NAME, opcode=row, uops=lower(spec, ver=ver), rd1_en=True
            ).sha(ver)
        except Exception:
            pass
    op = DveOp(NAME, spec, subdim=False, uops_sha=shas)
    dve_ops.OPS.append(op)
    dve_ops._SUB_OPCODE_FOR_NAME[NAME] = row
    dve_ops.CUSTOM_DVE_SPECS[NAME] = spec
    _GATE_OP = op
    return op


# --------------------------------------------------------------------------
# device kernel
# --------------------------------------------------------------------------

def _build_program():
    import concourse.bass as bass
    import concourse.mybir as mybir
    import concourse.tile as tile

    _install_tile_patch()
    dt = mybir.dt
    AF = mybir.ActivationFunctionType
    OP = mybir.AluOpType
    AX = mybir.AxisListType
    mm_dt = {"f32r": dt.float32r, "f32": dt.float32, "bf16": dt.bfloat16}[MM_DTYPE]
    f16 = dt.bfloat16

    nc = bass.Bass()
    xt_d = nc.declare_dram_parameter("xt", [S, G * BC], mm_dt, isOutput=False)
    w_d = nc.declare_dram_parameter("w", [G, S, M * 2 * H], mm_dt, isOutput=False)
    l_d = nc.declare_dram_parameter("l", [G, S, M * 256], mm_dt, isOutput=False)
    y_d = nc.declare_dram_parameter("y", [BC, G * H], f16, isOutput=True)

    n_blk = G // GRP
    UPB = GRP * NBC          # units per block (20)

    with tile.TileContext(nc) as tc:
        with (
            tc.tile_pool(name="xpool", bufs=1) as xpool,
            tc.tile_pool(name="wpool", bufs=7) as wpool,
            tc.tile_pool(name="lpool", bufs=7) as lpool,
            tc.tile_pool(name="hpsum", bufs=2, space="PSUM") as hpsum,
            tc.tile_pool(name="zpsum", bufs=2, space="PSUM") as zpsum,
            tc.tile_pool(name="spool", bufs=3) as spool,
            tc.tile_pool(name="ppool", bufs=3) as ppool,
            tc.tile_pool(name="npool", bufs=3) as npool,
            tc.tile_pool(name="vpool", bufs=2) as vpool,
            tc.tile_pool(name="accpool", bufs=3) as accpool,
            tc.tile_pool(name="statpool", bufs=2) as statpool,
        ):
            xt_sb = xpool.tile([S, G * BC], mm_dt)
            nc.sync.dma_start(xt_sb[:], xt_d[:])
            eps_sb = xpool.tile([128, 1], dt.float32, tag="eps")
            nc.vector.memset(eps_sb[:], EPS_LN)

            for blk in range(n_blk):
                gs = [blk * GRP + i for i in range(GRP)]
                w_sbs = {}
                l_sbs = {}
                for g in gs:
                    w_sbs[g] = wpool.tile([S, M * 2 * H], mm_dt, tag="w", name=f"wsb{g}")
                    nc.sync.dma_start(w_sbs[g][:], w_d[g])
                    l_sbs[g] = lpool.tile([S, M * 256], mm_dt, tag="l", name=f"lsb{g}")
                    nc.sync.dma_start(l_sbs[g][:], l_d[g])

                def xch(g, bc):
                    return xt_sb[:, g * BC + bc * 128: g * BC + (bc + 1) * 128]

                units = [(g, bc) for g in gs for bc in range(NBC)]

                # ---- phase A: stats for the whole block ----
                SW = 2 * M * UPB  # 8 cols per unit
                ss_all = statpool.tile([128, SW], dt.float32, tag="ss")
                mu_all = statpool.tile([128, SW], dt.float32, tag="mu")
                for u, (g, bc) in enumerate(units):
                    za = zpsum.tile([128, 1024], dt.float32, tag="za")
                    nc.tensor.matmul(za[:, 0:408], xch(g, bc), l_sbs[g][:, 0:408])
                    nc.tensor.matmul(za[:, 512:912], xch(g, bc), l_sbs[g][:, 512:912])
                    # squares of the 8 z-chunks (contiguous 400 per half)
                    psq = ppool.tile([128, 2 * M * S], f16, tag="p")
                    nc.scalar.activation(
                        psq[:, 0:4 * S], za[:, 0:4 * S], AF.Square)
                    nc.scalar.activation(
                        psq[:, 4 * S:8 * S], za[:, 512:512 + 4 * S], AF.Square)
                    nc.vector.reduce_sum(
                        ss_all[:, u * 2 * M:(u + 1) * 2 * M],
                        psq[:].rearrange("p (q r) -> p q r", r=S),
                        axis=AX.X)
                    # mean cols ride in za cols 400:408 (order 2m+t)
                    nc.scalar.activation(
                        mu_all[:, u * 2 * M:(u + 1) * 2 * M], za[:, 400:408],
                        AF.Copy)

                # ---- block smalls: var, rs, nb ----
                musq = statpool.tile([128, SW], dt.float32, tag="musq")
                nc.scalar.activation(musq[:], mu_all[:], AF.Square)
                var = statpool.tile([128, SW], dt.float32, tag="var")
                nc.vector.scalar_tensor_tensor(
                    var[:], ss_all[:], 1.0 / H, musq[:],
                    op0=OP.mult, op1=OP.subtract)
                varc = statpool.tile([128, SW], dt.float32, tag="varc")
                nc.vector.tensor_scalar(varc[:], var[:], 0.0, None, op0=OP.max)
                sd = statpool.tile([128, SW], dt.float32, tag="sd")
                nc.scalar.activation(sd[:], varc[:], AF.Sqrt, bias=eps_sb[:])
                rs = statpool.tile([128, SW], dt.float32, tag="rs")
                nc.vector.reciprocal(rs[:], sd[:])
                nb = statpool.tile([128, SW], dt.float32, tag="nb")
                nc.vector.scalar_tensor_tensor(
                    nb[:], mu_all[:], -1.0, rs[:], op0=OP.mult, op1=OP.mult)

                # ---- phase B: main matmuls + epilogue ----
                for u, (g, bc) in enumerate(units):
                    vt = vpool.tile([128, M, H], f16, tag="v")
                    for m in range(M):
                        hp = hpsum.tile([128, 2 * H], dt.float32, tag="h")
                        nc.tensor.matmul(
                            hp[:, 0:H], xch(g, bc),
                            w_sbs[g][:, m * 2 * H: m * 2 * H + H])
                        nc.tensor.matmul(
                            hp[:, H:2 * H], xch(g, bc),
                            w_sbs[g][:, m * 2 * H + H:(m + 1) * 2 * H])
                        c1 = slice(u * 2 * M + 2 * m, u * 2 * M + 2 * m + 1)
                        c2 = slice(u * 2 * M + 2 * m + 1, u * 2 * M + 2 * m + 2)
                        s_sb = spool.tile([128, H], f16, tag="s")
                        nc.scalar.activation(
                            s_sb[:], hp[:, 0:H], AF.Sigmoid,
                            bias=nb[:, c1], scale=rs[:, c1])
                        # n2 = (h2 - mu2) * rs2 ; v = max(n2,0) * s
                        n2 = npool.tile([128, H], f16, tag="n2")
                        nc.vector.tensor_scalar(
                            n2[:], hp[:, H:2 * H], mu_all[:, c2], rs[:, c2],
                            op0=OP.subtract, op1=OP.mult)
                        nc.vector.scalar_tensor_tensor(
                            vt[:, m], n2[:], 0.0, s_sb[:],
                            op0=OP.max, op1=OP.mult)
                    # mask-sum on gpsimd, all bf16 (Pool cost is byte-driven)
                    w01 = accpool.tile([128, H], f16, tag="w01")
                    nc.gpsimd.tensor_add(w01[:], vt[:, 0], vt[:, 1])
                    w23 = accpool.tile([128, H], f16, tag="w23")
                    nc.gpsimd.tensor_add(w23[:], vt[:, 2], vt[:, 3])
                    acc = accpool.tile([128, H], f16, tag="acc")
                    nc.gpsimd.tensor_add(acc[:], w01[:], w23[:])
                    nc.sync.dma_start(
                        y_d[bc * 128:(bc + 1) * 128, g * H:(g + 1) * H], acc[:])

    return nc


def _get_state():
    if "nc" not in _STATE:
        _STATE["nc"] = _build_program()
    return _STATE["nc"]


# --------------------------------------------------------------------------
# public entry point
# --------------------------------------------------------------------------

LAST_RESULTS = None


def kernel(x, W_masks, W1, W2, ln1_w, ln1_b, ln2_w, ln2_b):
    global LAST_RESULTS
    import ml_dtypes
    from concourse.bass_utils import run_bass_kernel_spmd

    assert np.allclose(np.asarray(ln1_w), 1.0) and np.allclose(np.asarray(ln2_w), 1.0) \
        and np.allclose(np.asarray(ln1_b), 0.0) and np.allclose(np.asarray(ln2_b), 0.0), \
        "kernel compiled for identity layernorm affine params"

    xt_cores, W_rhs, L_rhs = _host_prep(x, W_masks, W1, W2)
    np_dt = {"f32r": np.float32, "f32": np.float32,
             "bf16": ml_dtypes.bfloat16}[MM_DTYPE]
    W_rhs = W_rhs.astype(np_dt)
    L_rhs = L_rhs.astype(np_dt)

    nc = _get_state()
    in_maps = [
        {"xt": xt_cores[c].astype(np_dt), "w": W_rhs, "l": L_rhs}
        for c in range(N_CORES)
    ]
    res = run_bass_kernel_spmd(nc, in_maps, list(range(N_CORES)))
    LAST_RESULTS = res
    out = np.concatenate([res.results[c]["y"] for c in range(N_CORES)], axis=0)
    return out.astype(np.float32)


# revision 12
# speedup vs baseline: 1.0533x; 1.0533x over previous
"""Trainium2 Bass kernel for nn_DynamicFeatureGroupingLayer.

Reference computation (B=4096, G=10 groups of S=100 features, M=4 masks,
H=512 hidden):
    mask = entmax(1.1, W_masks)                       # [G,M,S]
    h_t[b,g,m,:] = (x_g[b] * mask[g,m]) @ W_t[g].T    # t in {1,2}
    n_t = layernorm(h_t)
    out[b,g] = sum_m relu(sigmoid(n_1) * n_2)         # [B, G*H]

Strategy:
  * Data-parallel over batch across 8 cores (512 rows each).
  * Host folds the mask into the weights: W~_t[g,m] = mask[g,m,:] * W_t[g]
    so h_t = x_g @ W~_t.T is a plain matmul (K=S=100, stationary = x chunk).
  * LN means come free as extra matmul columns (sum_h W~/H).
  * LN second moments via a Cholesky-Gram trick: ss = ||L^T x||^2 with
    L = chol(W~^T W~) [S,S]; stats run in x-space (S=100), 4x cheaper than
    h-space (H=512). Sum-of-squares via DVE bn_stats (even/odd mean+M2
    per 100-col chunk), recombined in the per-block smalls.
  * Epilogue per (group, mask): one ACT Sigmoid with per-partition
    scale/bias (rs1, nb1=-mu1*rs1), then a single custom DVE op
    GATE = relu(h2*rs2 + nb2) * s  ==  sigmoid(n1) * relu(n2)
    (valid since s > 0); mask-sum as 3 bf16 adds on GPSIMD; y is bf16.
"""

import numpy as np

B = 4096
INPUT_SIZE = 1000
H = 512
M = 4
S = 100
G = 10
N_CORES = 8
BC = B // N_CORES            # batch rows per core (512)
NBC = BC // 128              # 128-row chunks per core (4)
GRP = 5                      # groups g per block
EPS_LN = 1e-5

MM_DTYPE = "bf16"

_STATE = {}


# --------------------------------------------------------------------------
# host-side preprocessing
# --------------------------------------------------------------------------

def _entmax(alpha, v):
    v = v - np.max(v, axis=-1, keepdims=True)
    e = np.exp(v)
    s = (np.sum(e ** alpha, axis=-1, keepdims=True) + 1e-5) ** (1.0 / alpha)
    return e / s


def _host_prep(x, W_masks, W1, W2):
    """Returns (xt_per_core, W_rhs, L_rhs) as float32 arrays."""
    x = np.asarray(x, np.float32)
    mask = _entmax(1.1, np.asarray(W_masks, np.float64)).astype(np.float64)
    W1 = np.asarray(W1, np.float64)
    W2 = np.asarray(W2, np.float64)

    # W~_t[g,m,h,s] = mask[g,m,s] * W_t[g,h,s]
    Wt1 = mask[:, :, None, :] * W1[:, None, :, :]        # [G,M,H,S]
    Wt2 = mask[:, :, None, :] * W2[:, None, :, :]
    # main rhs: [G, S, M*2*H], col = m*1024 + t*512 + h
    W_rhs = np.stack([Wt1, Wt2], axis=2)                  # [G,M,2,H,S]
    W_rhs = W_rhs.transpose(0, 4, 1, 2, 3).reshape(G, S, M * 2 * H)

    # mean cols: value = sum_h W~/H; ride in spare tail cols 228:236 of the
    # m=0 block of L_rhs, order 2m+t
    MU = np.stack([Wt1.mean(axis=2), Wt2.mean(axis=2)], axis=2)  # [G,M,2,S]
    MU_rhs = MU.transpose(0, 3, 1, 2).reshape(G, S, 2 * M)

    # cholesky of gram matrices, tight-packed: chunk mt = m*2+t at col
    # (mt%4)*100 + (mt//4)*512; mu cols at 400:408 of the first half
    L_rhs = np.zeros((G, S, M * 256), np.float64)
    for g in range(G):
        for m in range(M):
            for t, Wt in enumerate((Wt1, Wt2)):
                Wm = Wt[g, m]                              # [H,S]
                Gm = Wm.T @ Wm                             # [S,S]
                jit = 1e-9 * np.trace(Gm) / S
                Lm = np.linalg.cholesky(Gm + jit * np.eye(S))
                mt = m * 2 + t
                base = (mt % 4) * 100 + (mt // 4) * 512
                L_rhs[g, :, base:base + S] = Lm
    L_rhs[:, :, 400:408] = MU_rhs

    # x transposed per core: xt[s, g*512 + b] = x[c*512+b, g*100+s]
    xt_cores = []
    for c in range(N_CORES):
        xc = x[c * BC:(c + 1) * BC]                        # [512, 1000]
        xt = np.ascontiguousarray(
            xc.reshape(BC, G, S).transpose(2, 1, 0).reshape(S, G * BC))
        xt_cores.append(xt)

    return xt_cores, W_rhs.astype(np.float32), L_rhs.astype(np.float32)


# --------------------------------------------------------------------------
# tile patch (this walrus build accepts at most ONE sync wait per inst)
# --------------------------------------------------------------------------

def _install_tile_patch():
    import concourse.mybir as mybir
    from concourse.tile import TileContext, ScopedClock

    if getattr(TileContext, "_drain_patched", False):
        return

    def _patched(self, tick_clock, wait_clock):
        nc = self.nc
        probe = nc.sync.nop(hint="drain_waits", nofuse=True)
        wait_clock.add_sem_waits(
            probe.ins, ScopedClock({None: tick_clock.global_clock}))
        si = probe.ins.sync_info
        if si is not None and len(si.on_wait) > 1:
            waits = list(si.on_wait)
            si.on_wait = [waits[0]]
            probe.ins.sync_info = si
            for w in waits[1:]:
                extra = nc.sync.nop(hint="drain_waits_x", nofuse=True)
                extra.ins.sync_info = mybir.SyncInfo(on_wait=[w], on_update=[])
        nc.sync.drain()
        nc.all_engine_barrier()
        popped = nc._tile_sem_poison_stack.pop()
        assert popped is self._sem_poison
        nc.clear_and_free_semaphores(list(self.sems.allocated().values()))
        nc.all_engine_barrier()

    TileContext._drain_and_barrier = _patched

    orig_commit = TileContext._commit_instruction

    def _commit_split(self, inst, lazy_reg_writes=True):
        si = inst.sync_info
        if (
            si is not None
            and len(si.on_wait) > 1
            and inst.engine != mybir.EngineType.Unassigned
        ):
            waits = list(si.on_wait)
            for w in waits[:-1]:
                nop = mybir.InstNoOp(
                    name=self.nc.get_next_instruction_name(),
                    engine=inst.engine,
                    ins=[],
                    outs=[],
                    sync_info=mybir.SyncInfo(on_wait=[w], on_update=[]),
                )
                orig_commit(self, nop, lazy_reg_writes=False)
            si.on_wait = [waits[-1]]
            inst.sync_info = si
        return orig_commit(self, inst, lazy_reg_writes)

    TileContext._commit_instruction = _commit_split
    TileContext._drain_patched = True


# --------------------------------------------------------------------------
# custom DVE op: GATE = relu(Src0*C0 + C1) * Src1
# (= sigmoid(n1) * relu(layernorm(h2)) with C0=rs2, C1=-mu2*rs2, Src1=s)
# --------------------------------------------------------------------------

_GATE_OP = None


def _register_gate_op():
    global _GATE_OP
    if _GATE_OP is not None:
        return _GATE_OP
    import concourse.dve_ops as dve_ops
    from concourse.dve_ops import DveOp, _dve_relu, _CUSTOM_DVE_ROW_BASE
    from concourse.dve_spec import C0, C1, Spec, Src0, Src1, lower, relu
    from concourse.dve_uop import DveOpSpec

    NAME = "TENSOR_GATE_LNRELU"
    for op in dve_ops.OPS:
        if op.name == NAME:
            _GATE_OP = op
            return op

    spec = Spec(
        body=relu(Src0 * C0 + C1) * Src1,
        reference=lambda in0, in1, c0, c1, c2: (
            _dve_relu(in0.astype(np.float32) * c0 + c1) * in1
        ),
    )
    row = _CUSTOM_DVE_ROW_BASE + len(dve_ops.OPS)
    shas = {}
    for ver in ("v3", "v4"):
        try:
            shas[ver] = DveOpSpec(
                name=NAME, opcode=row, uops=lower(spec, ver=ver), rd1_en=True
            ).sha(ver)
        except Exception:
            pass
    op = DveOp(NAME, spec, subdim=False, uops_sha=shas)
    dve_ops.OPS.append(op)
    dve_ops._SUB_OPCODE_FOR_NAME[NAME] = row
    dve_ops.CUSTOM_DVE_SPECS[NAME] = spec
    _GATE_OP = op
    return op


# --------------------------------------------------------------------------
# device kernel
# --------------------------------------------------------------------------

def _build_program():
    import concourse.bass as bass
    import concourse.mybir as mybir
    import concourse.tile as tile

    _install_tile_patch()
    dt = mybir.dt
    AF = mybir.ActivationFunctionType
    OP = mybir.AluOpType
    AX = mybir.AxisListType
    mm_dt = {"f32r": dt.float32r, "f32": dt.float32, "bf16": dt.bfloat16}[MM_DTYPE]
    f16 = dt.bfloat16

    nc = bass.Bass()
    xt_d = nc.declare_dram_parameter("xt", [S, G * BC], mm_dt, isOutput=False)
    w_d = nc.declare_dram_parameter("w", [G, S, M * 2 * H], mm_dt, isOutput=False)
    l_d = nc.declare_dram_parameter("l", [G, S, M * 256], mm_dt, isOutput=False)
    y_d = nc.declare_dram_parameter("y", [BC, G * H], f16, isOutput=True)

    n_blk = G // GRP
    UPB = GRP * NBC          # units per block (20)

    with tile.TileContext(nc) as tc:
        with (
            tc.tile_pool(name="xpool", bufs=1) as xpool,
            tc.tile_pool(name="wpool", bufs=7) as wpool,
            tc.tile_pool(name="lpool", bufs=7) as lpool,
            tc.tile_pool(name="hpsum", bufs=2, space="PSUM") as hpsum,
            tc.tile_pool(name="zpsum", bufs=2, space="PSUM") as zpsum,
            tc.tile_pool(name="spool", bufs=3) as spool,
            tc.tile_pool(name="ppool", bufs=3) as ppool,
            tc.tile_pool(name="npool", bufs=3) as npool,
            tc.tile_pool(name="vpool", bufs=2) as vpool,
            tc.tile_pool(name="accpool", bufs=3) as accpool,
            tc.tile_pool(name="statpool", bufs=2) as statpool,
        ):
            xt_sb = xpool.tile([S, G * BC], mm_dt)
            nc.sync.dma_start(xt_sb[:], xt_d[:])
            eps_sb = xpool.tile([128, 1], dt.float32, tag="eps")
            nc.vector.memset(eps_sb[:], EPS_LN)

            SW = 2 * M * UPB  # 8 stat cols per unit

            def xch(g, bc):
                return xt_sb[:, g * BC + bc * 128: g * BC + (bc + 1) * 128]

            def units_of(blk):
                return [(blk * GRP + i, bc)
                        for i in range(GRP) for bc in range(NBC)]

            wl_sbs = {}

            def load_weights(blk):
                for g in [blk * GRP + i for i in range(GRP)]:
                    w = wpool.tile([S, M * 2 * H], mm_dt, tag="w", name=f"wsb{g}")
                    nc.sync.dma_start(w[:], w_d[g])
                    l = lpool.tile([S, M * 256], mm_dt, tag="l", name=f"lsb{g}")
                    nc.sync.dma_start(l[:], l_d[g])
                    wl_sbs[g] = (w, l)

            stats = {}

            def alloc_stats(blk):
                ss = statpool.tile([128, SW], dt.float32, tag="ss")
                mu = statpool.tile([128, SW], dt.float32, tag="mu")
                stats[blk] = {"ss": ss, "mu": mu}

            def emit_A_unit(blk, u):
                g, bc = units_of(blk)[u]
                st = stats[blk]
                l_sb = wl_sbs[g][1]
                za = zpsum.tile([128, 1024], dt.float32, tag="za")
                nc.tensor.matmul(za[:, 0:408], xch(g, bc), l_sb[:, 0:408])
                nc.tensor.matmul(za[:, 512:912], xch(g, bc), l_sb[:, 512:912])
                psq = ppool.tile([128, 2 * M * S], f16, tag="p")
                nc.scalar.activation(psq[:, 0:4 * S], za[:, 0:4 * S], AF.Square)
                nc.scalar.activation(
                    psq[:, 4 * S:8 * S], za[:, 512:512 + 4 * S], AF.Square)
                nc.vector.reduce_sum(
                    st["ss"][:, u * 2 * M:(u + 1) * 2 * M],
                    psq[:].rearrange("p (q r) -> p q r", r=S),
                    axis=AX.X)
                # mean cols ride in za cols 400:408 (order 2m+t)
                nc.scalar.activation(
                    st["mu"][:, u * 2 * M:(u + 1) * 2 * M], za[:, 400:408],
                    AF.Copy)

            def emit_smalls(blk):
                st = stats[blk]
                musq = statpool.tile([128, SW], dt.float32, tag="musq")
                nc.scalar.activation(musq[:], st["mu"][:], AF.Square)
                var = statpool.tile([128, SW], dt.float32, tag="var")
                nc.vector.scalar_tensor_tensor(
                    var[:], st["ss"][:], 1.0 / H, musq[:],
                    op0=OP.mult, op1=OP.subtract)
                varc = statpool.tile([128, SW], dt.float32, tag="varc")
                nc.vector.tensor_scalar(varc[:], var[:], 0.0, None, op0=OP.max)
                sd = statpool.tile([128, SW], dt.float32, tag="sd")
                nc.scalar.activation(sd[:], varc[:], AF.Sqrt, bias=eps_sb[:])
                rs = statpool.tile([128, SW], dt.float32, tag="rs")
                nc.vector.reciprocal(rs[:], sd[:])
                nb = statpool.tile([128, SW], dt.float32, tag="nb")
                nc.vector.scalar_tensor_tensor(
                    nb[:], st["mu"][:], -1.0, rs[:], op0=OP.mult, op1=OP.mult)
                st["rs"] = rs
                st["nb"] = nb

            def emit_B_unit(blk, u):
                g, bc = units_of(blk)[u]
                st = stats[blk]
                w_sb = wl_sbs[g][0]
                mu_all, rs, nb = st["mu"], st["rs"], st["nb"]
                vt = vpool.tile([128, M, H], f16, tag="v")
                for m in range(M):
                    hp = hpsum.tile([128, 2 * H], dt.float32, tag="h")
                    nc.tensor.matmul(
                        hp[:, 0:H], xch(g, bc),
                        w_sb[:, m * 2 * H: m * 2 * H + H])
                    nc.tensor.matmul(
                        hp[:, H:2 * H], xch(g, bc),
                        w_sb[:, m * 2 * H + H:(m + 1) * 2 * H])
                    c1 = slice(u * 2 * M + 2 * m, u * 2 * M + 2 * m + 1)
                    c2 = slice(u * 2 * M + 2 * m + 1, u * 2 * M + 2 * m + 2)
                    s_sb = spool.tile([128, H], f16, tag="s")
                    nc.scalar.activation(
                        s_sb[:], hp[:, 0:H], AF.Sigmoid,
                        bias=nb[:, c1], scale=rs[:, c1])
                    # n2 = (h2 - mu2) * rs2 ; v = max(n2,0) * s
                    n2 = npool.tile([128, H], f16, tag="n2")
                    nc.vector.tensor_scalar(
                        n2[:], hp[:, H:2 * H], mu_all[:, c2], rs[:, c2],
                        op0=OP.subtract, op1=OP.mult)
                    nc.vector.scalar_tensor_tensor(
                        vt[:, m], n2[:], 0.0, s_sb[:],
                        op0=OP.max, op1=OP.mult)
                # mask-sum on gpsimd, all bf16 (Pool cost is byte-driven)
                w01 = accpool.tile([128, H], f16, tag="w01")
                nc.gpsimd.tensor_add(w01[:], vt[:, 0], vt[:, 1])
                w23 = accpool.tile([128, H], f16, tag="w23")
                nc.gpsimd.tensor_add(w23[:], vt[:, 2], vt[:, 3])
                acc = accpool.tile([128, H], f16, tag="acc")
                nc.gpsimd.tensor_add(acc[:], w01[:], w23[:])
                nc.sync.dma_start(
                    y_d[bc * 128:(bc + 1) * 128, g * H:(g + 1) * H], acc[:])

            # ---- software pipeline: A(k+1) interleaved into B(k) ----
            load_weights(0)
            alloc_stats(0)
            for u in range(UPB):
                emit_A_unit(0, u)
            emit_smalls(0)
            for blk in range(n_blk):
                nxt = blk + 1
                if nxt < n_blk:
                    load_weights(nxt)
                    alloc_stats(nxt)
                for u in range(UPB):
                    emit_B_unit(blk, u)
                    if nxt < n_blk:
                        emit_A_unit(nxt, u)
                if nxt < n_blk:
                    emit_smalls(nxt)

    return nc


def _get_state():
    if "nc" not in _STATE:
        _STATE["nc"] = _build_program()
    return _STATE["nc"]


# --------------------------------------------------------------------------
# public entry point
# --------------------------------------------------------------------------

LAST_RESULTS = None


def kernel(x, W_masks, W1, W2, ln1_w, ln1_b, ln2_w, ln2_b):
    global LAST_RESULTS
    import ml_dtypes
    from concourse.bass_utils import run_bass_kernel_spmd

    assert np.allclose(np.asarray(ln1_w), 1.0) and np.allclose(np.asarray(ln2_w), 1.0) \
        and np.allclose(np.asarray(ln1_b), 0.0) and np.allclose(np.asarray(ln2_b), 0.0), \
        "kernel compiled for identity layernorm affine params"

    xt_cores, W_rhs, L_rhs = _host_prep(x, W_masks, W1, W2)
    np_dt = {"f32r": np.float32, "f32": np.float32,
             "bf16": ml_dtypes.bfloat16}[MM_DTYPE]
    W_rhs = W_rhs.astype(np_dt)
    L_rhs = L_rhs.astype(np_dt)

    nc = _get_state()
    in_maps = [
        {"xt": xt_cores[c].astype(np_dt), "w": W_rhs, "l": L_rhs}
        for c in range(N_CORES)
    ]
    res = run_bass_kernel_spmd(nc, in_maps, list(range(N_CORES)))
    LAST_RESULTS = res
    out = np.concatenate([res.results[c]["y"] for c in range(N_CORES)], axis=0)
    return out.astype(np.float32)
